# revision 1
# baseline (speedup 1.0000x reference)
"""Trainium2 Bass kernel for nn_Blur: upfirdn2d(up=2, k=4x4 separable binomial).

Math: per (n,c) plane X [128,128] the output is out = A.T @ X @ A with
A [128,255] the 1D polyphase upsampling matrix (2 taps per output row).

Layout insight (from HW benchmarks): output DMA must write large contiguous
per-partition runs, so PLANES live on the partition dim at output time.
Per 128-plane window:
  - H-pass on PE: one fp32 matmul per input column w and y-half:
      psum[g, y] = X[:, :, w].T @ A'[:, yhalf]   (lhsT = X cols, M=planes)
  - ACT drains whole psum banks into S[g, w, y] (SBUF).
  - W-pass on DVE: two fused scalar_tensor_tensor ops per 16-row chunk:
      out[g, y, 2j]   = S[g,j,y] + r*S[g,j+1,y]
      out[g, y, 2j+1] = r*S[g,j,y] + S[g,j+1,y]     (r = v3/v1, v1 folded in A)
    plus x=254 boundary on GPSIMD.
  - Output DMA: [g, 16y, 255x] -> per-partition contiguous ~16KB runs
    (335 GB/s measured vs 41 GB/s for the naive y-on-partition layout).
Sharding: pure data parallel over batch, 2 images (256 planes) per core.
"""

import math

import numpy as np

import concourse.bacc as bacc
import concourse.mybir as mybir
import concourse.tile as tile
from concourse.bass_utils import run_bass_kernel_spmd

N_CORES = 8
N, C, H, W = 16, 128, 128, 128
HO = 2 * H - 1  # 255
PLANES_PER_CORE = (N // N_CORES) * C  # 256
WINDOW = 128  # planes per window (= output DMA partition span)
QLEN = 16  # output rows per staging tile / DMA
DT = mybir.dt.float32


def _taps_from_kernel(kernel2d: np.ndarray) -> np.ndarray:
    """Recover the 1D taps v (kernel2d == outer(v, v))."""
    k = np.asarray(kernel2d, dtype=np.float64)
    assert k.shape == (4, 4)
    v0 = math.sqrt(k[0, 0])
    v = k[0] / v0
    assert np.allclose(np.outer(v, v), k, rtol=1e-6), "kernel is not rank-1"
    assert abs(v[0] - v[3]) < 1e-12 and abs(v[1] - v[2]) < 1e-12, (
        "kernel taps not symmetric"
    )
    return v


def _build_amat(v: np.ndarray) -> np.ndarray:
    """A' = v1 * A, where A [128, 255] maps input rows to upsampled rows."""
    A = np.zeros((H, HO), dtype=np.float64)
    for y in range(HO):
        if y % 2 == 0:
            r = y // 2
            A[r, y] += v[1]
            if r + 1 < H:
                A[r + 1, y] += v[3]
        else:
            A[(y - 1) // 2, y] += v[0]
            A[(y + 1) // 2, y] += v[2]
    return (v[1] * A).astype(np.float32)


def _chunks(total: int, step: int):
    return [(s, min(step, total - s)) for s in range(0, total, step)]


def _build_bass(ratio: float, loop: int = 1, internal_out: bool = False):
    """Trace + compile the per-core Tile program. ratio = v3/v1."""
    nc = bacc.Bacc(
        "TRN2", target_bir_lowering=False, debug=False, num_devices=N_CORES
    )
    amat_d = nc.dram_tensor("amat", [H, HO], DT, kind="ExternalInput")
    if internal_out:
        # timing-only build: no big tensors cross the host link
        imgs_d = nc.dram_tensor("imgs_t", [PLANES_PER_CORE, H, W], DT)
        out_d = nc.dram_tensor("out", [PLANES_PER_CORE, HO, HO], DT)
        done_d = nc.dram_tensor("done", [1, 4], DT, kind="ExternalOutput")
    else:
        imgs_d = nc.dram_tensor(
            "imgs", [PLANES_PER_CORE, H, W], DT, kind="ExternalInput"
        )
        out_d = nc.dram_tensor(
            "out", [PLANES_PER_CORE, HO, HO], DT, kind="ExternalOutput"
        )
        done_d = None

    mult = mybir.AluOpType.mult
    add = mybir.AluOpType.add

    with tile.TileContext(nc) as tc:
        with (
            tc.tile_pool(name="const", bufs=1) as const_pool,
            tc.tile_pool(name="xin", bufs=1) as in_pool,
            tc.tile_pool(name="psum", bufs=4, space="PSUM") as psum_pool,
            tc.tile_pool(name="sblk", bufs=1) as s_pool,
            tc.tile_pool(name="outp", bufs=3) as out_pool,
        ):
            a1 = const_pool.tile([H, 128], DT)
            a2 = const_pool.tile([H, 127], DT)
            nc.sync.dma_start(a1[:], amat_d[:, 0:128])
            nc.sync.dma_start(a2[:], amat_d[:, 128:HO])

            def half_body(g0, x, y0, ylen, ach, win):
                # S stored [g, y, w]: stt APs get 4-8B inner strides
                s = s_pool.tile([128, 128, W], DT, tag="s")
                for wb in range(W // 8):
                    ps = psum_pool.tile([128, 8, 128], DT, tag="ps")
                    for wi in range(8):
                        w = 8 * wb + wi
                        nc.tensor.matmul(
                            ps[:, wi, 0:ylen],
                            x[:, :, w],
                            ach[:, 0:ylen],
                            start=True,
                            stop=True,
                        )
                    nc.scalar.copy(
                        s[:, 0:ylen, 8 * wb : 8 * wb + 8],
                        ps[:, :, 0:ylen].transpose([0, 2, 1]),
                    )

                for qs, qlen in _chunks(ylen, QLEN):
                    o = out_pool.tile([128, QLEN, HO], DT, tag="o")
                    q = slice(qs, qs + qlen)
                    sq0 = s[:, q, 0:127]
                    sq1 = s[:, q, 1:128]
                    # x = 2j   (j=0..126):   S[j] + r*S[j+1]
                    # x = 2j+1 (j=0..126): r*S[j] +   S[j+1]
                    nc.vector.scalar_tensor_tensor(
                        o[:, 0:qlen, 0:253:2],
                        sq1, ratio, sq0, op0=mult, op1=add,
                    )
                    nc.vector.scalar_tensor_tensor(
                        o[:, 0:qlen, 1:254:2],
                        sq0, ratio, sq1, op0=mult, op1=add,
                    )
                    # boundary x = 254: S[127]
                    nc.gpsimd.tensor_copy(o[:, 0:qlen, 254], s[:, q, 127])
                    dst = out_d[g0 : g0 + WINDOW]
                    nc.sync.dma_start(
                        dst[:, y0 + qs : y0 + qs + qlen, :],
                        o[:, 0:qlen, :],
                    )

            def window_body(win):
                g0 = win * WINDOW
                x = in_pool.tile([H, WINDOW, W], DT, tag="x")
                for k in range(WINDOW // 16):
                    src = imgs_d[g0 + 16 * k : g0 + 16 * (k + 1)]
                    # SWDGE path: separate queue from the SP-issued out DMAs
                    nc.gpsimd.dma_start(
                        x[:, 16 * k : 16 * (k + 1), :],
                        src.rearrange("g h w -> h g w"),
                    )
                for (y0, ylen), ach in (((0, 128), a1), ((128, 127), a2)):
                    half_body(g0, x, y0, ylen, ach, win)

            def full_body():
                for win in range(PLANES_PER_CORE // WINDOW):
                    window_body(win)

            if loop == 1:
                full_body()
            else:
                with tc.For_i(0, loop) as _:
                    full_body()

            if done_d is not None:
                nc.sync.dma_start(done_d[:], a1[0:1, 0:4])

    nc.compile()
    return nc


_CACHE: dict = {}


def _get_bass(kernel2d: np.ndarray):
    key = np.asarray(kernel2d, dtype=np.float32).tobytes()
    if key not in _CACHE:
        v = _taps_from_kernel(kernel2d)
        amat = _build_amat(v)
        ratio = float(v[3] / v[1])
        _CACHE[key] = (_build_bass(ratio), amat)
    return _CACHE[key]


def run(imgs: np.ndarray, kernel: np.ndarray, **spmd_kwargs):
    """Run on 8 NeuronCores; returns (full_output, BassKernelResults)."""
    imgs = np.ascontiguousarray(np.asarray(imgs, dtype=np.float32))
    assert imgs.shape == (N, C, H, W)
    nc, amat = _get_bass(kernel)

    per = N // N_CORES
    in_maps = [
        {
            "imgs": imgs[i * per : (i + 1) * per].reshape(
                PLANES_PER_CORE, H, W
            ),
            "amat": amat,
        }
        for i in range(N_CORES)
    ]
    res = run_bass_kernel_spmd(nc, in_maps, list(range(N_CORES)), **spmd_kwargs)
    out = np.concatenate(
        [r["out"].reshape(per, C, HO, HO) for r in res.results], axis=0
    )
    return out, res


def kernel(imgs: np.ndarray, kernel: np.ndarray) -> np.ndarray:
    out, _ = run(imgs, kernel)
    return out



# revision 3
# speedup vs baseline: 1.1198x; 1.1198x over previous
"""Trainium2 Bass kernel for nn_Blur: upfirdn2d(up=2, k=4x4 separable binomial).

Math: per (n,c) plane X [128,128] the output is out = A.T @ X @ A with
A [128,255] the 1D polyphase upsampling matrix (2 taps per output row).

Pipelined bf16-staging design (v1):
  - Input DMA (SWDGE/gpsimd): imgs fp32 -> x bf16 [h, g, w] cast during DMA.
    One 8.4MB DMA per 128-plane window; per-partition 512B runs.
  - H-pass on PE: one bf16 matmul per input column w, rhs = full A' [128, 255]
    (both y-halves merged, N=255): psum[g, wi, y] += x[:, :, w].T @ A'.
  - ACT drains each psum block twice (y 0:128 -> s1, y 128:255 -> s2) with
    fp32->bf16 cast and [g, y, w] transpose.
  - W-pass on DVE: s tiles are 129 wide with col 128 zeroed, so the x=254
    boundary folds into the even-x scalar_tensor_tensor:
      out[g, y, 2j]   = S[g,y,j] + r*S[g,y,j+1]   (j=0..127, S[128]=0)
      out[g, y, 2j+1] = r*S[g,y,j] + S[g,y,j+1]   (j=0..126, r=v3/v1)
  - Output DMA (HWDGE/sync): [128g, 16y, 255x] chunks -> per-partition
    contiguous ~16KB runs in HBM.
  All stages double/triple buffered: x bufs=2, s bufs=3, o bufs=2, psum
  bufs=2 so input DMA / PE / ACT / DVE / output DMA overlap across windows.
Sharding: pure data parallel over batch, 2 images (256 planes) per core.
"""

import math

import numpy as np

import concourse.bacc as bacc
import concourse.mybir as mybir
import concourse.tile as tile
from concourse.bass_utils import run_bass_kernel_spmd

N_CORES = 8
N, C, H, W = 16, 128, 128, 128
HO = 2 * H - 1  # 255
PLANES_PER_CORE = (N // N_CORES) * C  # 256
WINDOW = 128  # planes per window (= output DMA partition span)
QLEN = 16  # output rows per staging tile / DMA
DT = mybir.dt.float32
BF = mybir.dt.bfloat16


def _taps_from_kernel(kernel2d: np.ndarray) -> np.ndarray:
    """Recover the 1D taps v (kernel2d == outer(v, v))."""
    k = np.asarray(kernel2d, dtype=np.float64)
    assert k.shape == (4, 4)
    v0 = math.sqrt(k[0, 0])
    v = k[0] / v0
    assert np.allclose(np.outer(v, v), k, rtol=1e-6), "kernel is not rank-1"
    assert abs(v[0] - v[3]) < 1e-12 and abs(v[1] - v[2]) < 1e-12, (
        "kernel taps not symmetric"
    )
    return v


def _build_amat(v: np.ndarray) -> np.ndarray:
    """A' = v1 * A, where A [128, 255] maps input rows to upsampled rows."""
    A = np.zeros((H, HO), dtype=np.float64)
    for y in range(HO):
        if y % 2 == 0:
            r = y // 2
            A[r, y] += v[1]
            if r + 1 < H:
                A[r + 1, y] += v[3]
        else:
            A[(y - 1) // 2, y] += v[0]
            A[(y + 1) // 2, y] += v[2]
    return (v[1] * A).astype(np.float32)


def _chunks(total: int, step: int):
    return [(s, min(step, total - s)) for s in range(0, total, step)]


def _build_bass(ratio: float, loop: int = 1, internal_out: bool = False):
    """Trace + compile the per-core Tile program. ratio = v3/v1."""
    nc = bacc.Bacc(
        "TRN2", target_bir_lowering=False, debug=False, num_devices=N_CORES
    )
    amat_d = nc.dram_tensor("amat", [H, HO], DT, kind="ExternalInput")
    if internal_out:
        # timing-only build: no big tensors cross the host link
        imgs_d = nc.dram_tensor("imgs_t", [PLANES_PER_CORE, H, W], DT)
        out_d = nc.dram_tensor("out", [PLANES_PER_CORE, HO, HO], DT)
        done_d = nc.dram_tensor("done", [1, 4], DT, kind="ExternalOutput")
    else:
        imgs_d = nc.dram_tensor(
            "imgs", [PLANES_PER_CORE, H, W], DT, kind="ExternalInput"
        )
        out_d = nc.dram_tensor(
            "out", [PLANES_PER_CORE, HO, HO], DT, kind="ExternalOutput"
        )
        done_d = None

    mult = mybir.AluOpType.mult
    add = mybir.AluOpType.add

    with tile.TileContext(nc) as tc:
        with (
            tc.tile_pool(name="const", bufs=1) as const_pool,
            tc.tile_pool(name="xin", bufs=2) as in_pool,
            tc.tile_pool(name="psum", bufs=2, space="PSUM") as psum_pool,
            tc.tile_pool(name="sblk", bufs=3) as s_pool,
            tc.tile_pool(name="outp", bufs=2) as out_pool,
        ):
            a = const_pool.tile([H, HO], BF)
            # fp32 -> bf16 cast during DMA (SWDGE only)
            nc.gpsimd.dma_start(a[:], amat_d[:])

            def window_body(win):
                g0 = win * WINDOW
                x = in_pool.tile([H, WINDOW, W], BF, tag="x")
                for k in range(2):
                    gl, gh = 64 * k, 64 * (k + 1)
                    src = imgs_d[g0 + gl : g0 + gh]
                    nc.gpsimd.dma_start(
                        x[:, gl:gh, :], src.rearrange("g h w -> h g w")
                    )

                s1 = s_pool.tile([128, H, W + 1], BF, tag="s")
                s2 = s_pool.tile([128, H, W + 1], BF, tag="s")
                nc.vector.memset(s1[:, :, W], 0.0)
                nc.vector.memset(s2[:, 0 : HO - H, W], 0.0)

                for wb in range(W // 8):
                    ps = psum_pool.tile([128, 8, 256], DT, tag="ps")
                    for wi in range(8):
                        w = 8 * wb + wi
                        nc.tensor.matmul(
                            ps[:, wi, 0:HO],
                            x[:, :, w],
                            a[:],
                            start=True,
                            stop=True,
                        )
                    nc.scalar.copy(
                        s1[:, :, 8 * wb : 8 * wb + 8],
                        ps[:, :, 0:H].transpose([0, 2, 1]),
                    )
                    nc.scalar.copy(
                        s2[:, 0 : HO - H, 8 * wb : 8 * wb + 8],
                        ps[:, :, H:HO].transpose([0, 2, 1]),
                    )

                for y0, ylen, s in ((0, H, s1), (H, HO - H, s2)):
                    for qs, qlen in _chunks(ylen, QLEN):
                        o = out_pool.tile([128, QLEN, HO], DT, tag="o")
                        q = slice(qs, qs + qlen)
                        # x = 2j   (j=0..127):   S[j] + r*S[j+1]  (S[128]=0)
                        # x = 2j+1 (j=0..126): r*S[j] +   S[j+1]
                        nc.vector.scalar_tensor_tensor(
                            o[:, 0:qlen, 0:HO:2],
                            s[:, q, 1 : W + 1], ratio, s[:, q, 0:W],
                            op0=mult, op1=add,
                        )
                        nc.vector.scalar_tensor_tensor(
                            o[:, 0:qlen, 1 : HO - 1 : 2],
                            s[:, q, 0 : W - 1], ratio, s[:, q, 1:W],
                            op0=mult, op1=add,
                        )
                        dst = out_d[g0 : g0 + WINDOW]
                        nc.sync.dma_start(
                            dst[:, y0 + qs : y0 + qs + qlen, :],
                            o[:, 0:qlen, :],
                        )

            def full_body():
                for win in range(PLANES_PER_CORE // WINDOW):
                    window_body(win)

            if loop == 1:
                full_body()
            else:
                with tc.For_i(0, loop) as _:
                    full_body()

            if done_d is not None:
                nc.gpsimd.dma_start(done_d[:], a[0:1, 0:4])

    nc.compile()
    return nc


_CACHE: dict = {}


def _get_bass(kernel2d: np.ndarray):
    key = np.asarray(kernel2d, dtype=np.float32).tobytes()
    if key not in _CACHE:
        v = _taps_from_kernel(kernel2d)
        amat = _build_amat(v)
        ratio = float(v[3] / v[1])
        _CACHE[key] = (_build_bass(ratio), amat)
    return _CACHE[key]


def run(imgs: np.ndarray, kernel: np.ndarray, **spmd_kwargs):
    """Run on 8 NeuronCores; returns (full_output, BassKernelResults)."""
    imgs = np.ascontiguousarray(np.asarray(imgs, dtype=np.float32))
    assert imgs.shape == (N, C, H, W)
    nc, amat = _get_bass(kernel)

    per = N // N_CORES
    in_maps = [
        {
            "imgs": imgs[i * per : (i + 1) * per].reshape(
                PLANES_PER_CORE, H, W
            ),
            "amat": amat,
        }
        for i in range(N_CORES)
    ]
    res = run_bass_kernel_spmd(nc, in_maps, list(range(N_CORES)), **spmd_kwargs)
    out = np.concatenate(
        [r["out"].reshape(per, C, HO, HO) for r in res.results], axis=0
    )
    return out, res


def kernel(imgs: np.ndarray, kernel: np.ndarray) -> np.ndarray:
    out, _ = run(imgs, kernel)
    return out


# revision 4
# speedup vs baseline: 1.6972x; 1.5157x over previous
"""Trainium2 Bass kernel for nn_Blur: upfirdn2d(up=2, k=4x4 separable binomial).

The 4-tap up=2 blur is polyphase-decomposable: every output row (col) is a
2-tap FIR of two adjacent input rows (cols), with taps (v1,v3) for even and
(v3,v1) for odd phases (v = [1,3,3,1]/8, symmetric). So no matmul at all:

  - Input DMA (HWDGE/scalar ring): natural-layout [g, h, w] fp32 chunk loads,
    contiguous 8KB per-partition runs (planes on partitions). ~287 GB/s vs
    ~185-212 GB/s for the SWDGE h-major transposing load the PE path needs.
  - ACT: xp = v1^2 * x, fp32 -> bf16 (v1^2 = 9/64 and all tap ratios are
    exact in bf16; measured end-to-end rel err ~2.4e-3 from input rounding).
  - H-pass (DVE): s = v1*T via 2-tap stt along h:
      s[g, 2r,   w] = xp[g,r+1,w]*(v3/v1) + xp[g,r,w]
      s[g, 2r+1, w] = xp[g,r,w]*(v3/v1) + xp[g,r+1,w]
    xp has a zeroed row 128 so the y=254 boundary needs no special op.
  - W-pass (DVE): out = v1*s[j] + v3*s[j+1] via the same two stt forms; s
    tiles are 129 wide with col 128 zeroed so x=254 folds into the even stt.
  - Output DMA (HWDGE/sync ring): [128g, 16y, 255x] chunks -> contiguous
    ~16KB per-partition runs (~298 GB/s).
  Measured per-core DMA budget is ~297 GB/s combined (in+out share it), so
  the kernel is DMA-floor-bound at ~(16.8+66.6)MB / 297 GB/s ~= 280us; all
  compute (DVE ~51us/window, ACT ~14us/window) hides under the output DMA.
Sharding: pure data parallel over batch, 2 images (256 planes) per core.
"""

import math

import numpy as np

import concourse.bacc as bacc
import concourse.mybir as mybir
import concourse.tile as tile
from concourse.bass_utils import run_bass_kernel_spmd

N_CORES = 8
N, C, H, W = 16, 128, 128, 128
HO = 2 * H - 1  # 255
PLANES_PER_CORE = (N // N_CORES) * C  # 256
WINDOW = 128  # planes per window (= output DMA partition span)
QLEN = 16  # output rows per staging tile / DMA
DT = mybir.dt.float32
BF = mybir.dt.bfloat16


def _taps_from_kernel(kernel2d: np.ndarray) -> np.ndarray:
    """Recover the 1D taps v (kernel2d == outer(v, v))."""
    k = np.asarray(kernel2d, dtype=np.float64)
    assert k.shape == (4, 4)
    v0 = math.sqrt(k[0, 0])
    v = k[0] / v0
    assert np.allclose(np.outer(v, v), k, rtol=1e-6), "kernel is not rank-1"
    assert abs(v[0] - v[3]) < 1e-12 and abs(v[1] - v[2]) < 1e-12, (
        "kernel taps not symmetric"
    )
    return v


def _build_amat(v: np.ndarray) -> np.ndarray:
    """A' = v1 * A, where A [128, 255] maps input rows to upsampled rows.

    (Unused by the FIR kernel; kept for the host-side input contract.)"""
    A = np.zeros((H, HO), dtype=np.float64)
    for y in range(HO):
        if y % 2 == 0:
            r = y // 2
            A[r, y] += v[1]
            if r + 1 < H:
                A[r + 1, y] += v[3]
        else:
            A[(y - 1) // 2, y] += v[0]
            A[(y + 1) // 2, y] += v[2]
    return (v[1] * A).astype(np.float32)


def _chunks(total: int, step: int):
    return [(s, min(step, total - s)) for s in range(0, total, step)]


def _build_bass(
    ratio: float, loop: int = 1, internal_out: bool = False, v1sq: float = 9.0 / 64.0
):
    """Trace + compile the per-core Tile program. ratio = v3/v1."""
    nc = bacc.Bacc(
        "TRN2", target_bir_lowering=False, debug=False, num_devices=N_CORES
    )
    amat_d = nc.dram_tensor("amat", [H, HO], DT, kind="ExternalInput")
    if internal_out:
        # timing-only build: no big tensors cross the host link
        imgs_d = nc.dram_tensor("imgs_t", [PLANES_PER_CORE, H, W], DT)
        out_d = nc.dram_tensor("out", [PLANES_PER_CORE, HO, HO], DT)
        done_d = nc.dram_tensor("done", [1, 4], DT, kind="ExternalOutput")
    else:
        imgs_d = nc.dram_tensor(
            "imgs", [PLANES_PER_CORE, H, W], DT, kind="ExternalInput"
        )
        out_d = nc.dram_tensor(
            "out", [PLANES_PER_CORE, HO, HO], DT, kind="ExternalOutput"
        )
        done_d = None

    mult = mybir.AluOpType.mult
    add = mybir.AluOpType.add

    with tile.TileContext(nc) as tc:
        with (
            tc.tile_pool(name="const", bufs=1) as const_pool,
            tc.tile_pool(name="xc", bufs=2) as xc_pool,
            tc.tile_pool(name="xp", bufs=2) as xp_pool,
            tc.tile_pool(name="sblk", bufs=2) as s_pool,
            tc.tile_pool(name="outp", bufs=2) as out_pool,
        ):
            a = const_pool.tile([1, 16], DT)
            nc.sync.dma_start(a[:], amat_d[0:1, 0:16])

            stt = nc.vector.scalar_tensor_tensor

            def window_body(win):
                g0 = win * WINDOW
                xp = xp_pool.tile([128, H + 1, W], BF, tag="xp")
                nc.vector.memset(xp[:, H, :], 0.0)
                for k in range(8):
                    xc = xc_pool.tile([128, 16, W], DT, tag="xc")
                    src = imgs_d[g0 : g0 + WINDOW, 16 * k : 16 * (k + 1), :]
                    nc.scalar.dma_start(xc[:], src)
                    nc.scalar.mul(xp[:, 16 * k : 16 * (k + 1), :], xc[:], v1sq)

                s1 = s_pool.tile([128, H, W + 1], BF, tag="s")
                s2 = s_pool.tile([128, H, W + 1], BF, tag="s")
                nc.vector.memset(s1[:, :, W], 0.0)
                nc.vector.memset(s2[:, 0 : HO - H, W], 0.0)
                # H-pass: s = v1*T; T[2r] = v1*X[r]+v3*X[r+1], T[2r+1] = v3*X[r]+v1*X[r+1]
                stt(s1[:, 0:H:2, 0:W], xp[:, 1:65, :], ratio, xp[:, 0:64, :],
                    op0=mult, op1=add)
                stt(s1[:, 1:H:2, 0:W], xp[:, 0:64, :], ratio, xp[:, 1:65, :],
                    op0=mult, op1=add)
                stt(s2[:, 0 : HO - H : 2, 0:W], xp[:, 65 : H + 1, :], ratio,
                    xp[:, 64:H, :], op0=mult, op1=add)
                stt(s2[:, 1 : HO - H : 2, 0:W], xp[:, 64 : H - 1, :], ratio,
                    xp[:, 65:H, :], op0=mult, op1=add)

                for y0, ylen, s in ((0, H, s1), (H, HO - H, s2)):
                    for qs, qlen in _chunks(ylen, QLEN):
                        o = out_pool.tile([128, QLEN, HO], DT, tag="o")
                        q = slice(qs, qs + qlen)
                        # x = 2j   (j=0..127):   S[j] + r*S[j+1]  (S[128]=0)
                        # x = 2j+1 (j=0..126): r*S[j] +   S[j+1]
                        stt(o[:, 0:qlen, 0:HO:2],
                            s[:, q, 1 : W + 1], ratio, s[:, q, 0:W],
                            op0=mult, op1=add)
                        stt(o[:, 0:qlen, 1 : HO - 1 : 2],
                            s[:, q, 0 : W - 1], ratio, s[:, q, 1:W],
                            op0=mult, op1=add)
                        dst = out_d[g0 : g0 + WINDOW]
                        nc.sync.dma_start(
                            dst[:, y0 + qs : y0 + qs + qlen, :],
                            o[:, 0:qlen, :],
                        )

            def full_body():
                for win in range(PLANES_PER_CORE // WINDOW):
                    window_body(win)

            if loop == 1:
                full_body()
            else:
                with tc.For_i(0, loop) as _:
                    full_body()

            if done_d is not None:
                nc.sync.dma_start(done_d[:], a[0:1, 0:4])

    nc.compile()
    return nc


_CACHE: dict = {}


def _get_bass(kernel2d: np.ndarray):
    key = np.asarray(kernel2d, dtype=np.float32).tobytes()
    if key not in _CACHE:
        v = _taps_from_kernel(kernel2d)
        amat = _build_amat(v)
        ratio = float(v[3] / v[1])
        v1sq = float(v[1] * v[1])
        _CACHE[key] = (_build_bass(ratio, v1sq=v1sq), amat)
    return _CACHE[key]


def run(imgs: np.ndarray, kernel: np.ndarray, **spmd_kwargs):
    """Run on 8 NeuronCores; returns (full_output, BassKernelResults)."""
    imgs = np.ascontiguousarray(np.asarray(imgs, dtype=np.float32))
    assert imgs.shape == (N, C, H, W)
    nc, amat = _get_bass(kernel)

    per = N // N_CORES
    in_maps = [
        {
            "imgs": imgs[i * per : (i + 1) * per].reshape(
                PLANES_PER_CORE, H, W
            ),
            "amat": amat,
        }
        for i in range(N_CORES)
    ]
    res = run_bass_kernel_spmd(nc, in_maps, list(range(N_CORES)), **spmd_kwargs)
    out = np.concatenate(
        [r["out"].reshape(per, C, HO, HO) for r in res.results], axis=0
    )
    return out, res


def kernel(imgs: np.ndarray, kernel: np.ndarray) -> np.ndarray:
    out, _ = run(imgs, kernel)
    return out


# revision 7
# speedup vs baseline: 1.8532x; 1.0919x over previous
"""Trainium2 Bass kernel for nn_Blur: upfirdn2d(up=2, k=4x4 separable binomial).

The 4-tap up=2 blur is polyphase-separable: every output row (col) is a
2-tap FIR of two adjacent input rows (cols), with taps (v1,v3) for even and
(v3,v1) for odd phases (v = [1,3,3,1]/8, symmetric). No matmul at all:

  - Input DMA (HWDGE/scalar ring): natural-layout [g, h, w] fp32 chunk loads
    (planes on partitions, contiguous 8KB per-partition runs, ~287 GB/s --
    vs ~185-212 GB/s for the SWDGE h-major transposing load a PE path needs).
  - ACT: xp = v1^2 * x, fp32 -> bf16 (v1^2 = 9/64 and the tap ratio 1/3 are
    exact in bf16; end-to-end rel err ~2.4e-3, from input/staging rounding).
  - Per 32-row output chunk, all on DVE (s = v1*T staging, bf16):
      H-pass: s[2r-y0,   w] = xp[r+1,w]*(v3/v1) + xp[r,w]
              s[2r+1-y0, w] = xp[r,w]*(v3/v1) + xp[r+1,w]
      W-pass: out[y, 2j]   = s[y, j+1]*(v3/v1) + s[y, j]
              out[y, 2j+1] = s[y, j]*(v3/v1) + s[y, j+1]
    xp has a zeroed row 128 and s a zeroed col 128, so the y=254 / x=254
    boundaries need no special ops. Chunked s tiles (4KB) keep the first
    output DMA ~12us after kernel start and SBUF pressure low.
  - Output DMA (HWDGE/sync ring): [128g, 32y, 255x] fp32 chunks ->
    contiguous ~32KB per-partition runs (~7% faster than 16KB runs).
  Per-core DMA budget is ~300 GB/s combined (in+out share it), so the kernel
  is DMA-floor-bound at ~(16.8+66.6)MB / 300 GB/s ~= 280us; DVE (~100us) and
  ACT (~28us) hide under the output DMA.
Sharding: pure data parallel over batch, 2 images (256 planes) per core.
"""

import math

import numpy as np

import concourse.bacc as bacc
import concourse.mybir as mybir
import concourse.tile as tile
from concourse.bass_utils import run_bass_kernel_spmd

N_CORES = 8
N, C, H, W = 16, 128, 128, 128
HO = 2 * H - 1  # 255
PLANES_PER_CORE = (N // N_CORES) * C  # 256
WINDOW = 128  # planes per window (= output DMA partition span)
QLEN = 32  # output rows per staging tile / DMA
SW = W + 2  # s row width: col 128 = zero pad, col 129 = 4B-align pad
DT = mybir.dt.float32
BF = mybir.dt.bfloat16


def _taps_from_kernel(kernel2d: np.ndarray) -> np.ndarray:
    """Recover the 1D taps v (kernel2d == outer(v, v))."""
    k = np.asarray(kernel2d, dtype=np.float64)
    assert k.shape == (4, 4)
    v0 = math.sqrt(k[0, 0])
    v = k[0] / v0
    assert np.allclose(np.outer(v, v), k, rtol=1e-6), "kernel is not rank-1"
    assert abs(v[0] - v[3]) < 1e-12 and abs(v[1] - v[2]) < 1e-12, (
        "kernel taps not symmetric"
    )
    return v


def _build_amat(v: np.ndarray) -> np.ndarray:
    """A' = v1 * A, where A [128, 255] maps input rows to upsampled rows.

    (Unused on-device by the FIR kernel; kept for the host input contract.)"""
    A = np.zeros((H, HO), dtype=np.float64)
    for y in range(HO):
        if y % 2 == 0:
            r = y // 2
            A[r, y] += v[1]
            if r + 1 < H:
                A[r + 1, y] += v[3]
        else:
            A[(y - 1) // 2, y] += v[0]
            A[(y + 1) // 2, y] += v[2]
    return (v[1] * A).astype(np.float32)


def _chunks(total: int, step: int):
    return [(s, min(step, total - s)) for s in range(0, total, step)]


def _build_bass(
    ratio: float, loop: int = 1, internal_out: bool = False, v1sq: float = 9.0 / 64.0
):
    """Trace + compile the per-core Tile program. ratio = v3/v1."""
    nc = bacc.Bacc(
        "TRN2", target_bir_lowering=False, debug=False, num_devices=N_CORES
    )
    amat_d = nc.dram_tensor("amat", [H, HO], DT, kind="ExternalInput")
    if internal_out:
        # timing-only build: no big tensors cross the host link
        imgs_d = nc.dram_tensor("imgs_t", [PLANES_PER_CORE, H, W], DT)
        out_d = nc.dram_tensor("out", [PLANES_PER_CORE, HO, HO], DT)
        done_d = nc.dram_tensor("done", [1, 4], DT, kind="ExternalOutput")
    else:
        imgs_d = nc.dram_tensor(
            "imgs", [PLANES_PER_CORE, H, W], DT, kind="ExternalInput"
        )
        out_d = nc.dram_tensor(
            "out", [PLANES_PER_CORE, HO, HO], DT, kind="ExternalOutput"
        )
        done_d = None

    mult = mybir.AluOpType.mult
    add = mybir.AluOpType.add

    with tile.TileContext(nc) as tc:
        with (
            tc.tile_pool(name="const", bufs=1) as const_pool,
            tc.tile_pool(name="xc", bufs=2) as xc_pool,
            tc.tile_pool(name="xp", bufs=2) as xp_pool,
            tc.tile_pool(name="sblk", bufs=3) as s_pool,
            tc.tile_pool(name="outp", bufs=2) as out_pool,
        ):
            a = const_pool.tile([1, 16], DT)
            nc.sync.dma_start(a[:], amat_d[0:1, 0:16])

            stt = nc.vector.scalar_tensor_tensor

            def window_body(win):
                g0 = win * WINDOW
                xp = xp_pool.tile([128, H + 1, W], BF, tag="xp")
                nc.vector.memset(xp[:, H, :], 0.0)
                for k in range(4):
                    xc = xc_pool.tile([128, 32, W], DT, tag="xc")
                    h0 = 32 * k
                    src = imgs_d[g0 : g0 + WINDOW, h0 : h0 + 32, :]
                    nc.scalar.dma_start(xc[:], src)
                    for m in range(2):
                        nc.scalar.mul(
                            xp[:, h0 + 16 * m : h0 + 16 * (m + 1), :],
                            xc[:, 16 * m : 16 * (m + 1), :], v1sq)

                for y0, ylen in ((0, H), (H, HO - H)):
                    for qs, qlen in _chunks(ylen, QLEN):
                        r0 = (y0 + qs) // 2
                        ne = (qlen + 1) // 2
                        no = qlen // 2
                        sc = s_pool.tile([128, QLEN, SW], BF, tag="s")
                        nc.vector.memset(sc[:, 0:qlen, W], 0.0)
                        stt(sc[:, 0:qlen:2, 0:W],
                            xp[:, r0 + 1 : r0 + 1 + ne, :], ratio,
                            xp[:, r0 : r0 + ne, :], op0=mult, op1=add)
                        stt(sc[:, 1:qlen:2, 0:W],
                            xp[:, r0 : r0 + no, :], ratio,
                            xp[:, r0 + 1 : r0 + 1 + no, :], op0=mult, op1=add)
                        o = out_pool.tile([128, QLEN, HO], DT, tag="o")
                        stt(o[:, 0:qlen, 0:HO:2],
                            sc[:, 0:qlen, 1 : W + 1], ratio,
                            sc[:, 0:qlen, 0:W], op0=mult, op1=add)
                        stt(o[:, 0:qlen, 1 : HO - 1 : 2],
                            sc[:, 0:qlen, 0 : W - 1], ratio,
                            sc[:, 0:qlen, 1:W], op0=mult, op1=add)
                        dst = out_d[g0 : g0 + WINDOW]
                        nc.sync.dma_start(
                            dst[:, y0 + qs : y0 + qs + qlen, :],
                            o[:, 0:qlen, :],
                        )

            def full_body():
                for win in range(PLANES_PER_CORE // WINDOW):
                    window_body(win)

            if loop == 1:
                full_body()
            else:
                with tc.For_i(0, loop) as _:
                    full_body()

            if done_d is not None:
                nc.sync.dma_start(done_d[:], a[0:1, 0:4])

    nc.compile()
    return nc


_CACHE: dict = {}


def _get_bass(kernel2d: np.ndarray):
    key = np.asarray(kernel2d, dtype=np.float32).tobytes()
    if key not in _CACHE:
        v = _taps_from_kernel(kernel2d)
        amat = _build_amat(v)
        ratio = float(v[3] / v[1])
        v1sq = float(v[1] * v[1])
        _CACHE[key] = (_build_bass(ratio, v1sq=v1sq), amat)
    return _CACHE[key]


def run(imgs: np.ndarray, kernel: np.ndarray, **spmd_kwargs):
    """Run on 8 NeuronCores; returns (full_output, BassKernelResults)."""
    imgs = np.ascontiguousarray(np.asarray(imgs, dtype=np.float32))
    assert imgs.shape == (N, C, H, W)
    nc, amat = _get_bass(kernel)

    per = N // N_CORES
    in_maps = [
        {
            "imgs": imgs[i * per : (i + 1) * per].reshape(
                PLANES_PER_CORE, H, W
            ),
            "amat": amat,
        }
        for i in range(N_CORES)
    ]
    res = run_bass_kernel_spmd(nc, in_maps, list(range(N_CORES)), **spmd_kwargs)
    out = np.concatenate(
        [r["out"].reshape(per, C, HO, HO) for r in res.results], axis=0
    )
    return out, res


def kernel(imgs: np.ndarray, kernel: np.ndarray) -> np.ndarray:
    out, _ = run(imgs, kernel)
    return out
